# revision 20
# baseline (speedup 1.0000x reference)
"""Trainium2 Bass kernel for nn_Network_63763084476816 (GNN message passing).

The batched graph is structurally fixed: per graph, 38 clinical + 36 pixel
nodes, self-edges everywhere, and a complete bipartite pixel<->clinical edge
set.  Mean aggregation therefore collapses to dense math:

    h_c = relu(x_c @ (W_self + W_msg/37) + S_pix @ (W_msg/37) + b_g)
    h_p = relu(x_p @ (W_self + W_msg/39) + S_clin @ (W_msg/39) + b_g)
    gap = mean_p h_p
    out = relu([h_c | gap] @ W1 + b1) @ W2 + b2

Sharding: pure data parallel, 128 graphs per core on 8 cores; weights
(including W1) replicated.  Embeddings ship feature-major ([FV, node*BC+b])
so every matmul operand has its contraction dim on partitions.

The big streams (xt, W1) and the h-layer weights ship as bf16 (7.9
MB/core, half the fp32 traffic); bf16 keeps the PE on the 1-cycle/row
path at any N and halves LDWEIGHTS.  After that the kernel is PE-bound:
the tensor engine runs essentially back-to-back from the first usable xt
bytes to the last MLP chunk, so the design centers on feeding it early
and never letting it stall:

- DMA completion semaphores lag their bytes by ~2.5-3.5us (fixed HW
  completion latency), so consumers start ~3us after their data lands no
  matter how the queue is shaped.  xt segments are sized so each PSUM
  tile's columns get their own early semaphore.
- S_pix: each pixel half is folded to 2 blocks + carry by three idle-DVE
  adds, then 3 PE matmuls per half accumulate the sum in PSUM (identity
  stationary) during the window where the PE waits for clinical columns.
- S_clin: contiguous bf16 DVE tree-folds; the 20-block part-1 side folds
  completely while part 2 streams, keeping the post-arrival chain short.
- The aggregate matmuls read s4 through a stride-0 broadcast AP (the PE
  re-reads the same 128 columns), so no replication copies sit on the
  vector queue between the sum and its consumers.
- h PSUM tiles are 1024 cols (2 banks), one wide eviction each, spread
  over scalar (activation: bias+relu) and vector (tensor_scalar: +b_g,
  max 0); tiles align exactly with the W1 DMA groups.
- The scalar result [BC,1] is PE-transposed to one partition (b2 rides
  in as a K=1 matmul into the same accumulation group) so the store is a
  single 512B descriptor - a [BC,1] DMA shatters into BC 4-byte
  descriptors whose completion semaphore costs ~7us.

Accumulation stays fp32 in PSUM everywhere; W2 stays fp32.
"""

import sys

for _p in ("/opt/trn_rl_repo",):
    if _p not in sys.path:
        sys.path.insert(0, _p)

import numpy as np

_B = 1024
_NCORES = 8
_BC = _B // _NCORES  # 128 graphs per core
_NCLIN = 38
_NPIX = 36
_FV = 128
_HID = 512
_NCHUNK = 39  # K-chunks of 128 in the 4992-wide MLP contraction
# K-chunks per W1 DMA group; groups match the h PSUM tiles (8 chunks =
# 1024 cols = one 2-bank tile); last group tiny so the MLP tail after the
# final W1 arrival is one matmul.
_W1GROUPS = [8, 8, 8, 8, 6, 1]
_CCOLS = _NCLIN * _BC  # 4864
_PCOLS = _NPIX * _BC  # 4608

_CACHE = {}


def _build_bass():
    import concourse.bacc as bacc
    import concourse.mybir as mybir
    import concourse.tile as tile

    f32 = mybir.dt.float32
    f32r = mybir.dt.float32r
    bf16 = mybir.dt.bfloat16
    relu = mybir.ActivationFunctionType.Relu
    add_op = mybir.AluOpType.add
    max_op = mybir.AluOpType.max

    nc = bacc.Bacc("TRN2", target_bir_lowering=False, debug=False,
                   num_devices=_NCORES)

    xt_d = nc.dram_tensor("xt", [_FV, _CCOLS + _PCOLS], bf16, kind="ExternalInput")
    # W1 host-packed in the SBUF layout: [p, (chunk, n)] - long contiguous
    # per-partition runs for every DMA.
    w1_d = nc.dram_tensor("w1", [_FV, _NCHUNK * _HID], bf16, kind="ExternalInput")
    gw_d = nc.dram_tensor("gw", [_FV, 5 * _FV], bf16, kind="ExternalInput")
    aux_d = nc.dram_tensor("aux", [_BC, _HID + 3], f32, kind="ExternalInput")
    rowaux_d = nc.dram_tensor("rowaux", [1, _HID + 2 * _BC], bf16, kind="ExternalInput")
    ident_d = nc.dram_tensor("ident", [_FV, _FV], f32, kind="ExternalInput")
    out_d = nc.dram_tensor("out", [1, _BC], f32, kind="ExternalOutput")

    _LOWP = "bf16 operands/outputs; matmul accumulation stays fp32 in PSUM"

    with tile.TileContext(nc) as tc:
        with tc.tile_pool(name="main", bufs=1) as pool, \
             tc.tile_pool(name="hps", bufs=3, space="PSUM") as pps, \
             tc.tile_pool(name="zps", bufs=1, space="PSUM") as ppz:

            # Small parameter loads on the scalar (ACT) HWDGE ring so they
            # don't delay the big streams on the sync (SP) ring.
            gwsb = pool.tile([_FV, 5 * _FV], bf16, name="gwsb", tag="gwsb")
            nc.scalar.dma_start(gwsb[:], gw_d.ap())
            auxsb = pool.tile([_BC, _HID + 3], f32, name="auxsb", tag="auxsb")
            nc.scalar.dma_start(auxsb[:], aux_d.ap())
            rowsb = pool.tile([1, _HID + 2 * _BC], bf16, name="rowsb", tag="rowsb")
            nc.scalar.dma_start(rowsb[:], rowaux_d.ap())
            idsb = pool.tile([_FV, _FV], f32, name="idsb", tag="idsb")
            nc.scalar.dma_start(idsb[:], ident_d.ap())

            # Node embeddings, feature-major bf16.  Pixel section first (its
            # sum gates the clinical h blocks, which run first); both
            # sections in two parts so the S partial folds start before the
            # full section lands.  Clinical splits 20+18 blocks so each part
            # folds to an integral block count.
            xt = pool.tile([_FV, _CCOLS + _PCOLS], bf16, name="xt", tag="xt")
            _PH = _PCOLS // 2  # 2304 = 18 pixel blocks per half
            _CH = 20 * _BC  # 2560 = 20 clinical blocks
            nc.sync.dma_start(xt[:, _CCOLS:_CCOLS + _PH],
                              xt_d.ap()[:, _CCOLS:_CCOLS + _PH])
            nc.sync.dma_start(xt[:, _CCOLS + _PH:], xt_d.ap()[:, _CCOLS + _PH:])
            # clinical in four pieces, the first two aligned to the first h
            # tile's two x-matmuls (completion semaphores lag bytes by ~3us,
            # so each 512-col matmul gets its own early semaphore and the PE
            # stays fed - and p-state ramped - through the xt arrival window)
            nc.sync.dma_start(xt[:, :512], xt_d.ap()[:, :512])
            nc.sync.dma_start(xt[:, 512:_CH // 2], xt_d.ap()[:, 512:_CH // 2])
            nc.sync.dma_start(xt[:, _CH // 2:_CH], xt_d.ap()[:, _CH // 2:_CH])
            nc.sync.dma_start(xt[:, _CH:_CCOLS], xt_d.ap()[:, _CH:_CCOLS])

            # W1 streamed in groups; group g holds K-chunks as [FV, gch, HID].
            # W1 after xt on the same sync ring: FIFO order doubles as a
            # priority order, so the xt stream (which gates all compute)
            # never contends with the W1 stream.
            w1sb = [
                pool.tile([_FV, gch, _HID], bf16, name=f"w1sb{g}", tag=f"w1sb{g}")
                for g, gch in enumerate(_W1GROUPS)
            ]
            c0 = 0
            for g, gch in enumerate(_W1GROUPS):
                nc.sync.dma_start(
                    w1sb[g][:],
                    w1_d.ap()[:, c0 * _HID:(c0 + gch) * _HID].rearrange(
                        "p (c n) -> p c n", c=gch),
                )
                c0 += gch

            # ---- Per-graph node sums S[f, b].
            # S_pix on the PE: 36 N=128 accumulating matmuls with a bf16
            # identity stationary (out += I^T x_blk = x_blk), chasing the
            # pixel DMA pieces through the window where the PE would
            # otherwise idle waiting for clinical.  This replaces a serial
            # DVE fold chain whose tail gated the first aggregate matmuls.
            P0 = _CCOLS
            s4pix = pool.tile([_FV, _BC], bf16, name="s4pix", tag="s4pix")
            id_ap = gwsb[:, 4 * _FV:5 * _FV]
            sps = ppz.tile([_FV, _BC], f32, name="sps", tag="zz")
            # Per 18-block half: three DVE fold ops (vector is idle here)
            # squeeze the half to 2 blocks + 1 carry, so the PE accumulates
            # only 3 matmuls per half - the PE runs at its mid p-state this
            # early, so the fewer N=128 matmuls here the better.
            ppx = pool.tile([_FV, 2 * 1152], bf16, name="ppx", tag="ppx")
            pxb = pool.tile([_FV, 2 * 512], bf16, name="pxb", tag="pxb")
            pxc = pool.tile([_FV, 2 * 256], bf16, name="pxc", tag="pxc")
            nmm = 0
            for q in range(2):
                base = P0 + q * _PH
                a0 = q * 1152
                nc.vector.tensor_add(ppx[:, a0:a0 + 1152],
                                     xt[:, base:base + 1152],
                                     xt[:, base + 1152:base + 2304])
                nc.vector.tensor_add(pxb[:, q * 512:(q + 1) * 512],
                                     ppx[:, a0:a0 + 512],
                                     ppx[:, a0 + 512:a0 + 1024])
                nc.vector.tensor_add(pxc[:, q * 256:(q + 1) * 256],
                                     pxb[:, q * 512:q * 512 + 256],
                                     pxb[:, q * 512 + 256:(q + 1) * 512])
                for src in (pxc[:, q * 256:q * 256 + 128],
                            pxc[:, q * 256 + 128:(q + 1) * 256],
                            ppx[:, a0 + 1024:a0 + 1152]):
                    nc.tensor.matmul(sps[:], id_ap, src,
                                     start=(nmm == 0), stop=(nmm == 5))
                    nmm += 1
            with nc.allow_low_precision(reason=_LOWP):
                nc.vector.tensor_copy(s4pix[:], sps[:])

            # S_clin on the DVE: the 20-block part-1 side folds completely
            # to one block while part 2 streams; the post-part-2 chain is as
            # short as possible (the fold tail gates the pixel aggregate
            # matmuls).  All adds contiguous bf16 (2 elems/cycle).
            ucl = pool.tile([_FV, 1280], bf16, name="ucl", tag="ucl")
            wcl = pool.tile([_FV, 1024], bf16, name="wcl", tag="wcl")
            vcl = pool.tile([_FV, 1152], bf16, name="vcl", tag="vcl")
            xcl = pool.tile([_FV, 1024], bf16, name="xcl", tag="xcl")
            s4clin = pool.tile([_FV, _BC], bf16, name="s4clin", tag="s4clin")
            # part 1: 20 blocks -> ucl(10) -> 5 -> ... -> wcl[896:1024]
            nc.vector.tensor_add(ucl[:], xt[:, :1280], xt[:, 1280:2560])
            nc.vector.tensor_add(wcl[:, :640], ucl[:, :640], ucl[:, 640:1280])
            nc.vector.tensor_add(wcl[:, 640:896], wcl[:, :256], wcl[:, 256:512])
            nc.vector.tensor_add(wcl[:, 896:1024], wcl[:, 640:768],
                                 wcl[:, 768:896])

            def s4clin_tail():
                # part 2: 18 blocks -> 9 -> 4(+carry) -> 2 -> 1; then add the
                # part-1 total, part-1 carry (block idx 4 of the 5-fold), and
                # replicate.
                nc.vector.tensor_add(vcl[:], xt[:, 2560:3712], xt[:, 3712:4864])
                nc.vector.tensor_add(xcl[:, :512], vcl[:, :512], vcl[:, 512:1024])
                nc.vector.tensor_add(xcl[:, 512:768], xcl[:, :256],
                                     xcl[:, 256:512])
                nc.vector.tensor_add(xcl[:, 768:896], xcl[:, 512:640],
                                     xcl[:, 640:768])
                nc.vector.tensor_add(xcl[:, 896:1024], xcl[:, 768:896],
                                     vcl[:, 1024:1152])
                nc.vector.tensor_add(wcl[:, :128], wcl[:, 896:1024],
                                     wcl[:, 512:640])
                nc.vector.tensor_add(s4clin[:], wcl[:, :128],
                                     xcl[:, 896:1024])

            combT = pool.tile([_FV, _NCHUNK * _BC], bf16, name="combT", tag="combT")
            hpT = pool.tile([_FV, _PCOLS], bf16, name="hpT", tag="hpT")
            bg_ap = auxsb[:, _HID:_HID + 1]

            def evict(engine, dst, ps_ap):
                with nc.allow_low_precision(reason=_LOWP):
                    if engine == "s":
                        nc.scalar.activation(dst, ps_ap, relu, bias=bg_ap)
                    else:
                        # relu(ps + b_g): per-partition bias add then clamp
                        # at 0 - one DVE pass, PSUM -> bf16 SBUF.
                        nc.vector.tensor_scalar(
                            out=dst, in0=ps_ap, scalar1=bg_ap, scalar2=0.0,
                            op0=add_op, op1=max_op,
                        )

            def h_tile(width, a_ap, wm_ap, s4_ap, src0, dest, d0, name, eng,
                       pre_evict=None):
                # One 2-bank PSUM tile: up-to-512-col matmul pairs, then a
                # single wide eviction on the assigned engine.
                ps = pps.tile([_FV, width], f32, name=name, tag="hps")
                o = 0
                while o < width:
                    w = min(512, width - o)
                    nc.tensor.matmul(
                        ps[:, o:o + w], a_ap,
                        xt[:, src0 + d0 + o: src0 + d0 + o + w],
                        start=True, stop=False,
                    )
                    nc.tensor.matmul(
                        ps[:, o:o + w], wm_ap,
                        s4_ap.rearrange("p (o b) -> p o b", o=1)
                             .broadcast_to([_FV, w // _BC, _BC]),
                        start=False, stop=True,
                    )
                    o += w
                if pre_evict is not None:
                    pre_evict()  # vector-FIFO work that must precede this evict
                evict(eng, dest[:, d0:d0 + width], ps[:])

            # Clinical h -> combT blocks 0..37.  Tile widths match the W1
            # groups (8 chunks = 1024).  Eviction engines alternate so
            # scalar and vector drain PSUM concurrently; the S_clin fold
            # tail is spliced into the vector FIFO before the first
            # vector-owned eviction (both are ready around the same time,
            # and the fold gates the pixel s4 matmuls).
            a_c = gwsb[:, 0:_FV]
            wm_c = gwsb[:, 2 * _FV:3 * _FV]
            h_tile(1024, a_c, wm_c, s4pix[:], 0, combT, 0, "psc0", "v")
            h_tile(1024, a_c, wm_c, s4pix[:], 0, combT, 1024, "psc1", "s",
                   pre_evict=s4clin_tail)
            h_tile(1024, a_c, wm_c, s4pix[:], 0, combT, 2048, "psc2", "s")
            h_tile(1024, a_c, wm_c, s4pix[:], 0, combT, 3072, "psc3", "s")
            h_tile(768, a_c, wm_c, s4pix[:], 0, combT, 4096, "psc4", "s")

            # Pixel h -> hpT.
            a_p = gwsb[:, _FV:2 * _FV]
            wm_p = gwsb[:, 3 * _FV:4 * _FV]
            h_tile(1024, a_p, wm_p, s4clin[:], _CCOLS, hpT, 0, "psp0", "v")
            h_tile(1024, a_p, wm_p, s4clin[:], _CCOLS, hpT, 1024, "psp1", "s")
            h_tile(1024, a_p, wm_p, s4clin[:], _CCOLS, hpT, 2048, "psp2", "v")
            h_tile(1024, a_p, wm_p, s4clin[:], _CCOLS, hpT, 3072, "psp3", "s")
            h_tile(512, a_p, wm_p, s4clin[:], _CCOLS, hpT, 4096, "psp4", "v")

            # gap block (plain sum; the 1/36 is folded into W1's last rows).
            ugp = pool.tile([_FV, 2304], bf16, name="ugp", tag="ugp")
            vgp = pool.tile([_FV, 1152], bf16, name="vgp", tag="vgp")
            wgp = pool.tile([_FV, 896], bf16, name="wgp", tag="wgp")
            nc.vector.tensor_add(ugp[:, :1152], hpT[:, :1152], hpT[:, 1152:2304])
            nc.vector.tensor_add(ugp[:, 1152:], hpT[:, 2304:3456], hpT[:, 3456:4608])
            nc.vector.tensor_add(vgp[:], ugp[:, :1152], ugp[:, 1152:])
            nc.vector.tensor_add(wgp[:, :512], vgp[:, :512], vgp[:, 512:1024])
            nc.vector.tensor_add(wgp[:, 512:768], wgp[:, :256], wgp[:, 256:512])
            nc.vector.tensor_add(wgp[:, 768:896], wgp[:, 512:640], wgp[:, 640:768])
            nc.vector.tensor_add(combT[:, _NCLIN * _BC:], wgp[:, 768:896],
                                 vgp[:, 1024:1152])

            # MLP layer 1: psz[b, n] = sum_k combined[b, k] W1[k, n] (+ b1).
            # Emission order = PE FIFO order: early-arriving W1 groups first,
            # then the b1 matmul and the gap chunk (ready mid-stream), and the
            # last-arriving W1 groups at the end so nothing head-blocks.
            psz = ppz.tile([_BC, _HID], f32, name="psz", tag="zz")

            def mlp_chunk(k, start, stop):
                goff = 0
                for g, gch in enumerate(_W1GROUPS):
                    if k < goff + gch:
                        nc.tensor.matmul(
                            psz[:],
                            combT[:, k * _BC:(k + 1) * _BC],
                            w1sb[g][:, k - goff, :],
                            start=start, stop=stop,
                        )
                        return
                    goff += gch

            for k in range(32):  # groups 0-3 (chunks 0..31)
                mlp_chunk(k, start=(k == 0), stop=False)
            nc.tensor.matmul(psz[:], rowsb[:, _HID:_HID + _BC], rowsb[:, :_HID],
                             start=False, stop=False)  # + b1
            for k in range(32, 38):  # group 4
                mlp_chunk(k, start=False, stop=False)
            # chunk 38 = gap x W1 group 5: both the gap h-values and the last
            # W1 bytes are the latest to arrive, so this goes last.
            mlp_chunk(38, start=False, stop=True)

            # MLP layer 2 fused: one DVE op does relu (max with 0), the W2
            # multiply, and the free-dim sum, reading psz directly from PSUM.
            # (tensor_tensor_reduce wedges the device on this path;
            # scalar_tensor_tensor with accum_out is HW-verified.)
            zw = pool.tile([_BC, _HID], f32, name="zw", tag="zw")
            osum = pool.tile([_BC, 1], f32, name="osum", tag="osum")
            nc.vector.scalar_tensor_tensor(
                out=zw[:], in0=psz[:], scalar=0.0, in1=auxsb[:, :_HID],
                op0=mybir.AluOpType.max, op1=mybir.AluOpType.mult,
                accum_out=osum[:],
            )
            # Gather the per-partition scalars onto one partition (PE
            # transpose) so the output store is one contiguous descriptor;
            # b2 rides in as a K=1 matmul into the same accumulation group.
            pst = ppz.tile([1, _BC], f32, name="pst", tag="zz")
            nc.tensor.matmul(pst[:], osum[:], idsb[:], is_transpose=True,
                             start=True, stop=False)
            nc.tensor.matmul(pst[:], rowsb[:, _HID:_HID + 1],
                             rowsb[:, _HID + _BC:], start=False, stop=True)
            orow = pool.tile([1, _BC], f32, name="orow", tag="orow")
            nc.vector.tensor_copy(orow[:], pst[:])
            nc.scalar.dma_start(out_d.ap(), orow[:])

    nc.compile()
    return nc


def _host_prep(W_self, W_msg, b_g, W1, b1, W2, b2):
    import ml_dtypes

    f32 = np.float32
    bf16 = ml_dtypes.bfloat16
    wmc = np.asarray(W_msg, f32) / f32(37.0)
    wmp = np.asarray(W_msg, f32) / f32(39.0)
    ws = np.asarray(W_self, f32)
    gw = np.ascontiguousarray(
        np.hstack([ws + wmc, ws + wmp, wmc, wmp,
                   np.eye(_FV, dtype=f32)]).astype(bf16))
    w1m = np.array(W1, dtype=f32, copy=True)
    w1m[_NCLIN * _FV:, :] /= f32(_NPIX)
    # Pack to SBUF layout [p, (chunk, n)]: w1p[p, c*HID+n] = w1m[c*FV+p, n].
    w1m = np.ascontiguousarray(
        w1m.reshape(_NCHUNK, _FV, _HID).transpose(1, 0, 2).reshape(_FV, -1)
        .astype(bf16))
    aux = np.empty((_BC, _HID + 3), dtype=f32)
    aux[:, :_HID] = np.asarray(W2, f32).reshape(1, _HID)
    aux[:, _HID] = np.asarray(b_g, f32)
    aux[:, _HID + 1] = f32(np.asarray(b2, f32).reshape(-1)[0])
    aux[:, _HID + 2] = f32(0.0)
    rowaux = np.empty((1, _HID + 2 * _BC), dtype=bf16)
    rowaux[0, :_HID] = np.asarray(b1, f32).astype(bf16)
    rowaux[0, _HID:_HID + _BC] = bf16(1.0)
    rowaux[0, _HID + _BC:] = bf16(np.asarray(b2, f32).reshape(-1)[0])
    ident = np.eye(_FV, dtype=f32)
    return gw, w1m, aux, rowaux, ident


def _xt_for_core(clinical, image, k):
    import ml_dtypes

    bf16 = ml_dtypes.bfloat16
    sl = slice(k * _BC, (k + 1) * _BC)
    xc = np.ascontiguousarray(clinical[sl].transpose(2, 1, 0)).reshape(_FV, _CCOLS)
    xp = np.ascontiguousarray(image[sl].transpose(2, 1, 0)).reshape(_FV, _PCOLS)
    return np.ascontiguousarray(
        np.concatenate([xc, xp], axis=1).astype(bf16))


def kernel(**inputs):
    clinical = np.asarray(inputs["clinical_embeddings"], np.float32)
    image = np.asarray(inputs["image_embeddings"], np.float32)
    gw, w1m, aux, rowaux, ident = _host_prep(
        inputs["W_self"], inputs["W_msg"], inputs["b_g"],
        inputs["W1"], inputs["b1"], inputs["W2"], inputs["b2"],
    )

    if "nc" not in _CACHE:
        _CACHE["nc"] = _build_bass()
    nc = _CACHE["nc"]

    in_maps = [
        {
            "xt": _xt_for_core(clinical, image, k),
            "w1": w1m,
            "gw": gw,
            "aux": aux,
            "rowaux": rowaux,
            "ident": ident,
        }
        for k in range(_NCORES)
    ]

    from concourse.bass_utils import run_bass_kernel_spmd

    res = run_bass_kernel_spmd(
        nc, in_maps, core_ids=list(range(_NCORES)),
        trace=bool(_CACHE.get("trace", False)),
        **_CACHE.get("run_kwargs", {}),
    )
    _CACHE["last_results"] = res
    out = np.concatenate(
        [r["out"].reshape(_BC, 1) for r in res.results], axis=0)
    return np.ascontiguousarray(out.astype(np.float32))
